# revision 1
# baseline (speedup 1.0000x reference)
"""MultiHeadedAttention Trainium2 Bass kernel.

Full inputs in, full output out. Sharding: 8 cores = 4 batches x 2 head-pairs
(data-parallel over batch, tensor-parallel over the 4 heads). Per core, all
matmuls in bf16 (fp32 PSUM accumulation):
  Q/K projections for its 2 heads      -> [128, 2048] bf16 (chan-major)
  V projection directly transposed     -> vt [m, (h, d+ones)] bf16
  per head: scoresT[m,n] = K^T Q, exp on ACT (scale=1/8; no max-sub needed,
  |s/8| < ~5), x[d+1, n] accumulated over m in PSUM with vt as the stationary
  operand (ones row gives softmax sums), normalize via DMA-broadcast 1/sums,
  out projection with both heads accumulated in PSUM.
Host pre-casts inputs/weights to bf16, sums the two per-batch partials and
adds the output bias in fp32.
"""

import sys

if "/opt/trn_rl_repo" not in sys.path:
    sys.path.insert(0, "/opt/trn_rl_repo")

import numpy as np
import ml_dtypes

BF = ml_dtypes.bfloat16

B, D, N, H = 4, 256, 2048, 4
DIM = D // H  # 64
NW = 4  # 512-wide n windows
MB = 16  # 128-wide m blocks

_CACHE = {}


def _emit(ctx, tc, io):
    import concourse.bass as bass
    import concourse.mybir as mybir

    nc = tc.nc
    f32 = mybir.dt.float32
    bf16 = mybir.dt.bfloat16
    EXP = mybir.ActivationFunctionType.Exp
    LN = mybir.ActivationFunctionType.Ln

    const = ctx.enter_context(tc.tile_pool(name="const", bufs=1))
    xin = ctx.enter_context(tc.tile_pool(name="xin", bufs=4))
    big = ctx.enter_context(tc.tile_pool(name="big", bufs=1))
    xpool = ctx.enter_context(tc.tile_pool(name="xpool", bufs=2))
    pb = ctx.enter_context(tc.tile_pool(name="probs", bufs=3))
    work = ctx.enter_context(tc.tile_pool(name="work", bufs=2))
    outp = ctx.enter_context(tc.tile_pool(name="outp", bufs=3))
    psA = ctx.enter_context(tc.tile_pool(name="psA", bufs=2, space="PSUM"))
    psX = ctx.enter_context(tc.tile_pool(name="psX", bufs=4, space="PSUM"))
    dpool = ctx.enter_context(tc.tile_pool(name="dpool", bufs=2, space="DRAM"))

    # ---- constants / weights (all bf16 except the f32 Q/K biases) ----
    wqt_sb = const.tile([128, 2, 128], bf16, tag="wqt")
    nc.sync.dma_start(wqt_sb, io["wqt"].rearrange("(c p) o -> p c o", p=128))
    wkt_sb = const.tile([128, 2, 128], bf16, tag="wkt")
    nc.sync.dma_start(wkt_sb, io["wkt"].rearrange("(c p) o -> p c o", p=128))
    wvt_sb = const.tile([128, 2, 128], bf16, tag="wvt")
    nc.sync.dma_start(wvt_sb, io["wvt"].rearrange("(c p) o -> p c o", p=128))
    wmt0_sb = const.tile([64, 256], bf16, tag="wmt0")
    nc.sync.dma_start(wmt0_sb, io["wmt0"])
    wmt1_sb = const.tile([64, 256], bf16, tag="wmt1")
    nc.sync.dma_start(wmt1_sb, io["wmt1"])
    bq_sb = const.tile([128, 1], f32, tag="bq")
    nc.sync.dma_start(bq_sb, io["bq"])
    bk_sb = const.tile([128, 1], f32, tag="bk")
    nc.sync.dma_start(bk_sb, io["bk"])
    bv_sb = const.tile([1, 128], bf16, tag="bv")
    nc.sync.dma_start(bv_sb, io["bv"])
    onesb = const.tile([1, 128], bf16, tag="onesb")
    nc.gpsimd.memset(onesb, 1.0)
    ones64f = const.tile([1, 64], f32, tag="ones64f")
    nc.gpsimd.memset(ones64f, 1.0)

    # PE warmup: the HAM clock gate only releases (1.2 -> 2.4 GHz) after a
    # fully-busy ~3.4us window. Run a burst of back-to-back matmuls during the
    # input-DMA ramp so the attention phase starts (and stays) warm.
    wu_a = const.tile([128, 128], bf16, tag="wu_a")
    nc.gpsimd.memset(wu_a, 0.0)
    wu_b = const.tile([128, 512], bf16, tag="wu_b")
    nc.gpsimd.memset(wu_b, 0.0)
    wu_ps = psA.tile([128, 1024], f32, tag="ps", name="wu_ps")
    for i in range(48):
        nc.tensor.matmul(wu_ps[:, 0:512], lhsT=wu_a, rhs=wu_b, start=True, stop=True)

    # ---- input loads (chunked 2x per tile for DMA-queue spread, issued from
    # three different HWDGE engines so the rings run in parallel) ----
    xq_t, xk_t, xv_t = [], [], []
    eng = {"xq": nc.sync, "xk": nc.scalar, "xv": nc.sync}
    for w in range(NW):
        for name, lst in (("xq", xq_t), ("xk", xk_t), ("xv", xv_t)):
            t = xin.tile([128, 2, 512], bf16, tag=name, name=f"{name}{w}")
            src = io[name].rearrange("(c p) n -> p c n", p=128)
            for hh in range(2):
                s = slice(w * 512 + hh * 256, w * 512 + (hh + 1) * 256)
                eng[name].dma_start(t[:, :, hh * 256 : (hh + 1) * 256], src[:, :, s])
            lst.append(t)

    # ---- phases 1+2 interleaved: emit K-w0, Q-w0/w1 and the first four
    # V^T blocks first so the attention m-loop can start while the rest of
    # the projections and V^T blocks still stream in.
    q_sb = big.tile([128, 2048], bf16, tag="q")
    k_sb = big.tile([128, 2048], bf16, tag="k")

    def proj_step(xt, wt, bias, dst, w):
        ps = psA.tile([128, 1024], f32, tag="ps", name=f"psproj{w}")
        nc.tensor.matmul(ps[:, 0:512], lhsT=wt[:, 0, :], rhs=xt[w][:, 0, :], start=True, stop=False)
        nc.tensor.matmul(ps[:, 0:512], lhsT=wt[:, 1, :], rhs=xt[w][:, 1, :], start=False, stop=True)
        nc.vector.tensor_scalar_add(dst[:, w * 512 : (w + 1) * 512], ps[:, 0:512], bias)

    vt = big.tile([128, MB, 2, 65], bf16, tag="vt")
    nc.gpsimd.memset(vt[:, :, :, 64:65], 1.0)

    def vt_step(mb):
        w, off = divmod(mb, 4)
        ms = slice(off * 128, (off + 1) * 128)
        ps = psA.tile([128, 1024], f32, tag="ps", name=f"psvt{mb}")
        pvt = ps[:, 0:128]
        nc.tensor.matmul(pvt, lhsT=onesb, rhs=bv_sb, start=True, stop=False)
        nc.tensor.matmul(pvt, lhsT=xv_t[w][:, 0, ms], rhs=wvt_sb[:, 0, :], start=False, stop=False)
        nc.tensor.matmul(pvt, lhsT=xv_t[w][:, 1, ms], rhs=wvt_sb[:, 1, :], start=False, stop=True)
        nc.vector.tensor_copy(vt[:, mb, :, 0:64], pvt.rearrange("m (h d) -> m h d", h=2))

    proj_step(xk_t, wkt_sb, bk_sb, k_sb, 0)
    proj_step(xq_t, wqt_sb, bq_sb, q_sb, 0)
    proj_step(xq_t, wqt_sb, bq_sb, q_sb, 1)
    for mb in range(4):
        vt_step(mb)
    proj_step(xk_t, wkt_sb, bk_sb, k_sb, 1)
    proj_step(xq_t, wqt_sb, bq_sb, q_sb, 2)
    proj_step(xk_t, wkt_sb, bk_sb, k_sb, 2)
    proj_step(xq_t, wqt_sb, bq_sb, q_sb, 3)
    proj_step(xk_t, wkt_sb, bk_sb, k_sb, 3)
    for mb in range(4, MB):
        vt_step(mb)

    # ---- phase 3: attention per head ----
    # PE-order grouping: without explicit deps the scheduler alternates
    # scores and x-accum matmuls, forcing a LDWEIGHTS before every matmul.
    # Enforce [4 scores of mb+1][4 x-accums of mb] alternation instead.
    from concourse.tile_rust import add_dep_helper

    def _raw(inst):
        return getattr(inst, "ins", inst)

    x_sb = []
    sc_groups = []  # scores-matmul groups in emission order across heads/halves
    xa_groups = []
    for h in range(2):
        qh = q_sb[h * 64 : (h + 1) * 64, :]
        kh = k_sb[h * 64 : (h + 1) * 64, :]
        xh = xpool.tile([64, 2048], bf16, tag="x", name=f"x{h}")
        # n split in two 1024 halves: x-accum psum drops to 2 banks, which
        # lets the scores psum triple-buffer (3x2 + 2 = 8 banks) so the PE
        # never stalls on exp.
        for nh in range(2):
            nbase = nh * 1024
            px = [psX.tile([65, 512], f32, tag="px", name=f"px{h}_{nh}_{i}") for i in range(2)]
            for mb in range(MB):
                pt = pb.tile([128, 1024], bf16, tag="pt", name="pt")
                sc = psA.tile([128, 1024], f32, tag="ps", name="pssc")
                scg = []
                for s2 in range(2):
                    n0 = nbase + s2 * 512
                    scg.append(nc.tensor.matmul(
                        sc[:, s2 * 512 : (s2 + 1) * 512],
                        lhsT=kh[:, mb * 128 : (mb + 1) * 128],
                        rhs=qh[:, n0 : n0 + 512],
                        start=True,
                        stop=True,
                    ))
                nc.scalar.activation(pt, sc, EXP, scale=0.125)
                sc_groups.append(scg)
                xag = []
                for j in range(2):
                    xag.append(nc.tensor.matmul(
                        px[j],
                        lhsT=vt[:, mb, h, :],
                        rhs=pt[:, j * 512 : (j + 1) * 512],
                        start=(mb == 0),
                        stop=(mb == MB - 1),
                        skip_group_check=True,
                    ))
                # filler matmul: reuses the vt stationary (no LDWEIGHTS) and
                # writes a dead region of the just-consumed scores bank. Keeps
                # the PE's HAM activity window busy so the 2.4GHz clock gate
                # stays open once the warmup burst releases it.
                nc.tensor.matmul(
                    sc[0:65, 0:512],
                    lhsT=vt[:, mb, h, :],
                    rhs=wu_b,
                    start=True,
                    stop=True,
                    skip_group_check=True,
                )
                xa_groups.append(xag)

            # normalize this half: broadcast sums via DRAM bounce, then
            # multi-lane DVE reciprocal + multiply.
            s_row = work.tile([1, 1024], f32, tag="s_row", name=f"s_row{h}_{nh}")
            for j in range(2):
                nc.scalar.copy(s_row[:, j * 512 : (j + 1) * 512], px[j][64:65, :])
            s_dram = dpool.tile([1, 1024], f32, tag="s_dram", name=f"s_dram{h}_{nh}")
            nc.sync.dma_start(s_dram, s_row)
            s_bc = work.tile([64, 1024], f32, tag="s_bc", name=f"s_bc{h}_{nh}")
            s_src = bass.AP(
                tensor=s_dram.tensor,
                offset=s_dram.offset,
                ap=[[0, 64]] + list(s_dram.ap[1:]),
            )
            nc.sync.dma_start(s_bc, s_src)
            r_bc = work.tile([64, 1024], f32, tag="r_bc", name=f"r_bc{h}_{nh}")
            nc.vector.reciprocal(r_bc, s_bc)
            for j in range(2):
                nc.vector.tensor_mul(
                    xh[:, nbase + j * 512 : nbase + (j + 1) * 512],
                    px[j][0:64, :],
                    r_bc[:, j * 512 : (j + 1) * 512],
                )
        x_sb.append(xh)

    # PE alternation deps: xa[g] after sc[g+1]; sc[g+2] after xa[g]
    G = len(sc_groups)
    for g in range(G):
        if g + 1 < G:
            for m in xa_groups[g]:
                add_dep_helper(_raw(m), _raw(sc_groups[g + 1][-1]), False,
                               "group x-accums after next scores")
        if g + 2 < G:
            for m in sc_groups[g + 2]:
                add_dep_helper(_raw(m), _raw(xa_groups[g][-1]), False,
                               "group scores after prev x-accums")

    if "dbg_q" in io:
        nc.sync.dma_start(io["dbg_q"], q_sb)
        nc.sync.dma_start(io["dbg_k"], k_sb)
        nc.sync.dma_start(io["dbg_vt"], vt)
        nc.sync.dma_start(io["dbg_x0"], x_sb[0])
        nc.sync.dma_start(io["dbg_x1"], x_sb[1])

    # ---- phase 4: out projection, heads accumulated in PSUM ----
    for oc in range(2):
        ocs = slice(oc * 128, (oc + 1) * 128)
        po = [psA.tile([128, 1024], f32, tag="ps", name=f"po{oc}_{g}") for g in range(2)]
        for g in range(2):
            for s2 in range(2):
                w = g * 2 + s2
                nc.tensor.matmul(po[g][:, s2 * 512 : (s2 + 1) * 512], lhsT=wmt0_sb[:, ocs],
                                 rhs=x_sb[0][:, w * 512 : (w + 1) * 512], start=True, stop=False)
        for g in range(2):
            for s2 in range(2):
                w = g * 2 + s2
                nc.tensor.matmul(po[g][:, s2 * 512 : (s2 + 1) * 512], lhsT=wmt1_sb[:, ocs],
                                 rhs=x_sb[1][:, w * 512 : (w + 1) * 512], start=False, stop=True)
        for g in range(2):
            ws = slice(g * 1024, (g + 1) * 1024)
            ot = outp.tile([128, 1024], f32, tag="ot", name="ot")
            nc.vector.tensor_copy(ot, po[g])
            nc.sync.dma_start(io["out"][ocs, ws], ot)


def _build_nc(debug_dumps=False):
    key = ("nc", debug_dumps)
    if key in _CACHE:
        return _CACHE[key]
    from contextlib import ExitStack

    import concourse.mybir as mybir
    import concourse.tile as tile
    from concourse import bacc

    f32 = mybir.dt.float32
    bf16 = mybir.dt.bfloat16
    nc = bacc.Bacc("TRN2", target_bir_lowering=False, debug=False, num_devices=8)
    io = {}
    for name, shape, dt_ in (
        ("xq", [256, 2048], bf16),
        ("xk", [256, 2048], bf16),
        ("xv", [256, 2048], bf16),
        ("wqt", [256, 128], bf16),
        ("wkt", [256, 128], bf16),
        ("wvt", [256, 128], bf16),
        ("bq", [128, 1], f32),
        ("bk", [128, 1], f32),
        ("bv", [1, 128], bf16),
        ("wmt0", [64, 256], bf16),
        ("wmt1", [64, 256], bf16),
    ):
        io[name] = nc.dram_tensor(name, shape, dt_, kind="ExternalInput").ap()
    io["out"] = nc.dram_tensor("out", [256, 2048], f32, kind="ExternalOutput").ap()
    if debug_dumps:
        io["dbg_q"] = nc.dram_tensor("dbg_q", [128, 2048], bf16, kind="ExternalOutput").ap()
        io["dbg_k"] = nc.dram_tensor("dbg_k", [128, 2048], bf16, kind="ExternalOutput").ap()
        io["dbg_vt"] = nc.dram_tensor("dbg_vt", [128, MB, 2, 65], bf16, kind="ExternalOutput").ap()
        io["dbg_x0"] = nc.dram_tensor("dbg_x0", [64, 2048], bf16, kind="ExternalOutput").ap()
        io["dbg_x1"] = nc.dram_tensor("dbg_x1", [64, 2048], bf16, kind="ExternalOutput").ap()

    with tile.TileContext(nc) as tc:
        with ExitStack() as ctx:
            _emit(ctx, tc, io)
    nc.compile()
    _CACHE[key] = nc
    _CACHE[(key, "io")] = io
    return nc


def make_in_maps(query, key, value, wq, bq, wk, bk, wv, bv, wm, bm):
    fb = lambda a: np.ascontiguousarray(np.asarray(a, dtype=np.float32)).astype(BF)
    f = lambda a: np.ascontiguousarray(np.asarray(a), dtype=np.float32)
    query, key, value = f(query), f(key), f(value)
    wq, wk, wv, wm = f(wq), f(wk), f(wv), f(wm)
    bq, bk, bv = f(bq), f(bk), f(bv)
    in_maps = []
    for c in range(8):
        b, pair = divmod(c, 2)
        hs = (2 * pair, 2 * pair + 1)
        idx = np.array([d * H + h for h in hs for d in range(DIM)])
        m = {
            "xq": fb(query[b]),
            "xk": fb(key[b]),
            "xv": fb(value[b]),
            "wqt": fb(wq[idx].T),
            "wkt": fb(wk[idx].T),
            "wvt": fb(wv[idx].T),
            "bq": f(bq[idx].reshape(128, 1)),
            "bk": f(bk[idx].reshape(128, 1)),
            "bv": fb(bv[idx].reshape(1, 128)),
            "wmt0": fb(wm[:, idx[:64]].T),
            "wmt1": fb(wm[:, idx[64:]].T),
        }
        in_maps.append(m)
    return in_maps


def run(in_maps, trace=False, **kw):
    from concourse import bass_utils

    nc = _build_nc()
    return bass_utils.run_bass_kernel_spmd(
        nc, in_maps, core_ids=list(range(8)), trace=trace, **kw
    )


def gather(results, bm):
    bm = np.asarray(bm, dtype=np.float32)
    outs = [np.asarray(r["out"], dtype=np.float32) for r in results]
    return np.stack([outs[2 * b] + outs[2 * b + 1] + bm[:, None] for b in range(B)])


def kernel(query, key, value, wq, bq, wk, bk, wv, bv, wm, bm):
    in_maps = make_in_maps(query, key, value, wq, bq, wk, bk, wv, bv, wm, bm)
    res = run(in_maps)
    return gather(res.results, bm)



# revision 12
# speedup vs baseline: 1.3011x; 1.3011x over previous
"""MultiHeadedAttention Trainium2 Bass kernel (v2).

Full inputs in, full output out. Sharding: 8 cores = 4 batches x 2 head-pairs
(data-parallel over batch, tensor-parallel over the 4 heads).

Per core (batch b, heads h0/h1), all matmuls bf16 except the fp8 x-accum:
  - Q/K proj -> q_sb/k_sb [128 (h,d), 2048] bf16; bias folded into the
    mandatory PSUM->SBUF copy (ACT Identity with per-partition bias AP /
    DVE tensor_scalar_add).
  - V proj -> vt [128 m, 8 j, 2 h, 2 i, 80] fp8e4 (DoubleRow pair layout,
    i = m-block//8, pair (j, j+8); col 64 = ones row for softmax sums).
  - scores: row-tiled matmul pairs via tile_position (h0 rows 0:63,
    h1 rows 64:127) -> two adjacent PSUM banks [128, 1024].
  - exp: ONE instruction per (c4, mb) covering both heads' banks.
    ACT: exp(0.125*s + ln2) -> fp8e4.  DVE: Schraudolph bit-trick
    int8(s/ln2 + 63.8) bitcast as fp8e4 (same 2x scale; scale cancels in
    softmax). Alternating per pair-slot j.
  - x-accum: fp8 DoubleRow matmul per (h, j): K=256 (m-blocks j and j+8),
    M=65 (64 d + ones), N=512. PSUM accumulates over j; row 64 = sums.
  - normalize: copy px->SBUF, broadcast sums row via SBUF->SBUF DMA,
    reciprocal_approx_fast, multiply -> xcat [128 (h,d), 512] bf16.
  - out-proj: lhsT=wmcat [128 (h,d), 128 oc] (heads fused, K=128),
    streamed per 512-wide n chunk; out fp32, host adds pair partials + bias.
"""

import sys

if "/opt/trn_rl_repo" not in sys.path:
    sys.path.insert(0, "/opt/trn_rl_repo")

import numpy as np
import ml_dtypes

BF = ml_dtypes.bfloat16
F8 = ml_dtypes.float8_e4m3fn

B, D, N, H = 4, 256, 2048, 4
DIM = D // H  # 64
NW = 4   # 512-wide input windows
MB = 16  # 128-wide m blocks
LN2 = 0.6931471805599453
SCHRAUD_A = 1.0 / LN2          # bits = A*s + B  (score -> fp8e4 bit pattern)
SCHRAUD_B = 63.8               # 64 = x2 scale (matches ACT's +ln2 bias)

_CACHE = {}

import os
CFG_DR = os.environ.get("K_DR", "1") == "1"          # DoubleRow x-accum
CFG_SCHRAUD = os.environ.get("K_SCHRAUD", "1") == "1"  # DVE bit-trick exp
CFG_ACT_FP8 = os.environ.get("K_ACT_FP8", "1") == "1"  # ACT exp -> fp8 out
CFG_TILEPOS = os.environ.get("K_TILEPOS", "1") == "1"  # row-tiled scores


def _emit(ctx, tc, io):
    import concourse.bass as bass
    import concourse.mybir as mybir

    nc = tc.nc
    f32 = mybir.dt.float32
    bf16 = mybir.dt.bfloat16
    fp8 = mybir.dt.float8e4
    i8 = mybir.dt.int8
    EXP = mybir.ActivationFunctionType.Exp
    IDENT = mybir.ActivationFunctionType.Identity
    MUL = mybir.AluOpType.mult
    ADD = mybir.AluOpType.add
    DR = mybir.MatmulPerfMode.DoubleRow

    const = ctx.enter_context(tc.tile_pool(name="const", bufs=1))
    xin = ctx.enter_context(tc.tile_pool(name="xin", bufs=12))
    big = ctx.enter_context(tc.tile_pool(name="big", bufs=1))
    ptp = ctx.enter_context(tc.tile_pool(name="ptp", bufs=2))
    work = ctx.enter_context(tc.tile_pool(name="work", bufs=4))
    xcp = ctx.enter_context(tc.tile_pool(name="xcp", bufs=2))
    outp = ctx.enter_context(tc.tile_pool(name="outp", bufs=4))
    psSC = ctx.enter_context(tc.tile_pool(name="psSC", bufs=2, space="PSUM"))
    psPX = ctx.enter_context(tc.tile_pool(name="psPX", bufs=4, space="PSUM"))
    dpool = ctx.enter_context(tc.tile_pool(name="dpool", bufs=4, space="DRAM"))

    # ---- weights (scalar + gpsimd queues; small, first) ----
    wqt_sb = const.tile([128, 2, 128], bf16, tag="wqt")
    nc.scalar.dma_start(wqt_sb, io["wqt"])
    wkt_sb = const.tile([128, 2, 128], bf16, tag="wkt")
    nc.scalar.dma_start(wkt_sb, io["wkt"])
    bq_sb = const.tile([128, 1], f32, tag="bq")
    nc.scalar.dma_start(bq_sb, io["bq"])
    bk_sb = const.tile([128, 1], f32, tag="bk")
    nc.scalar.dma_start(bk_sb, io["bk"])
    wvt_sb = const.tile([128, 2, 128], bf16, tag="wvt")
    nc.gpsimd.dma_start(wvt_sb, io["wvt"])
    bv_sb = const.tile([1, 128], bf16, tag="bv")
    nc.gpsimd.dma_start(bv_sb, io["bv"])
    wmcat_sb = const.tile([128, 256], bf16, tag="wmcat")
    nc.gpsimd.dma_start(wmcat_sb, io["wmcat"])
    onesb = const.tile([1, 128], bf16, tag="onesb")
    nc.gpsimd.memset(onesb, 1.0)
    ln2b = const.tile([128, 1], f32, tag="ln2b")
    nc.gpsimd.memset(ln2b, LN2)

    # ---- PE warmup: garbage matmuls release the HAM clock gate while the
    # input DMAs stream. Values are never read. ----
    wu = const.tile([128, 512], bf16, tag="wu")
    nc.vector.memset(wu[0:1, 0:1], 0.0)  # allocate; rest is garbage, never read
    wu_ps = psSC.tile([128, 1024], f32, tag="sc", name="wu_ps")
    for _ in range(14):
        nc.tensor.matmul(wu_ps[:, 0:512], lhsT=wu[:, 0:128], rhs=wu,
                         start=True, stop=True)

    # ---- input DMAs: window-contiguous host layout, 2KB/partition per
    # window. K first (scores m loop), Q w0 (first n chunk), V next. ----
    xq_t, xk_t, xv_t = [None] * NW, [None] * NW, [None] * NW
    order = [("xk", 0), ("xq", 0), ("xv", 0), ("xk", 1), ("xv", 1),
             ("xk", 2), ("xv", 2), ("xk", 3), ("xv", 3),
             ("xq", 1), ("xq", 2), ("xq", 3)]
    tiles = {"xq": xq_t, "xk": xk_t, "xv": xv_t}
    engs = [nc.sync, nc.gpsimd]
    for n_i, (name, w) in enumerate(order):
        t = xin.tile([128, 2, 512], bf16, tag=name, name=f"{name}{w}")
        engs[n_i % 2].dma_start(t, io[name][:, w, :, :])
        tiles[name][w] = t

    # ---- vt tile + ones columns (col 64 of each (j, h, i) slot).
    # Cols 65:79 are zero pad so the DoubleRow weights AP can be the full
    # 80-byte (16-aligned) slice with M=80; output rows 65:79 are dead. ----
    vt = big.tile([128, 8, 2, 2, 80], fp8, tag="vt")
    nc.gpsimd.memset(vt, 0.0)
    for h in range(2):
        for i in range(2):
            nc.gpsimd.memset(vt[:, :, h, i, 64:65], 1.0)

    q_sb = big.tile([128, 2048], bf16, tag="q")
    k_sb = big.tile([128, 2048], bf16, tag="k")

    # ---- projections (PE emission order = DMA arrival order) ----
    def qk_proj(xt, wt, bias, dst, w, use_act):
        ps = psPX.tile([128, 512], f32, tag="px", name=f"ps_{dst.name}{w}")
        nc.tensor.matmul(ps, lhsT=wt[:, 0, :], rhs=xt[w][:, 0, :],
                         start=True, stop=False)
        nc.tensor.matmul(ps, lhsT=wt[:, 1, :], rhs=xt[w][:, 1, :],
                         start=False, stop=True)
        ws = slice(w * 512, (w + 1) * 512)
        if use_act:
            nc.scalar.activation(dst[:, ws], ps, IDENT, bias=bias, scale=1.0)
        else:
            nc.vector.tensor_scalar_add(dst[:, ws], ps, bias)

    def v_proj(w, use_act):
        # 4 m-blocks (mb = 4w+r) -> one [128, 512] psum -> one fp8 copy into
        # vt[:, 4*(w%2)+r, :, w//2, 0:64]
        ps = psPX.tile([128, 512], f32, tag="px", name=f"ps_v{w}")
        for r in range(4):
            mb = 4 * w + r
            ms = slice(mb * 128 - w * 512, mb * 128 - w * 512 + 128)
            pvt = ps[:, r * 128:(r + 1) * 128]
            nc.tensor.matmul(pvt, lhsT=onesb, rhs=bv_sb, start=True, stop=False)
            nc.tensor.matmul(pvt, lhsT=xv_t[w][:, 0, ms], rhs=wvt_sb[:, 0, :],
                             start=False, stop=False)
            nc.tensor.matmul(pvt, lhsT=xv_t[w][:, 1, ms], rhs=wvt_sb[:, 1, :],
                             start=False, stop=True)
        j0 = 4 * (w % 2)
        src = ps[:, :].rearrange("m (r h d) -> m r h d", r=4, h=2)
        dst = vt[:, j0:j0 + 4, :, w // 2, 0:64]
        if use_act:
            nc.scalar.copy(dst, src)
        else:
            nc.vector.tensor_copy(dst, src)

    qk_proj(xk_t, wkt_sb, bk_sb, k_sb, 0, True)
    qk_proj(xq_t, wqt_sb, bq_sb, q_sb, 0, False)
    v_proj(0, True)
    qk_proj(xk_t, wkt_sb, bk_sb, k_sb, 1, False)
    v_proj(1, False)
    qk_proj(xk_t, wkt_sb, bk_sb, k_sb, 2, True)
    v_proj(2, True)
    qk_proj(xk_t, wkt_sb, bk_sb, k_sb, 3, False)
    v_proj(3, False)
    qk_proj(xq_t, wqt_sb, bq_sb, q_sb, 1, True)
    qk_proj(xq_t, wqt_sb, bq_sb, q_sb, 2, False)
    qk_proj(xq_t, wqt_sb, bq_sb, q_sb, 3, True)

    # ---- attention: c4-outer (4 x 512-wide n chunks), mb inner ----
    # Tail work of chunk c4 is emitted interleaved into chunk c4+1's mb loop
    # to avoid head-of-line blocking on the in-order engines.
    deferred = {}  # mb position -> list of callables

    def run_deferred(mb):
        for fn in deferred.pop(mb, []):
            fn()

    def make_tail(c4, px, xcat):
        cs = slice(c4 * 512, (c4 + 1) * 512)
        state = {}

        def cp_step(h, use_act):
            def fn():
                cp = work.tile([65, 512], f32, tag="cp", name=f"cp{c4}_{h}")
                if use_act:
                    nc.scalar.copy(cp, px[h][0:65, :])
                else:
                    nc.vector.tensor_copy(cp, px[h][0:65, :])
                s_dram = dpool.tile([1, 512], f32, tag="s_dram",
                                    name=f"s_dram{c4}_{h}")
                nc.sync.dma_start(s_dram, cp[64:65, :])
                rb = work.tile([64, 512], f32, tag="rb", name=f"rb{c4}_{h}")
                src = bass.AP(tensor=s_dram.tensor, offset=s_dram.offset,
                              ap=[[0, 64]] + list(s_dram.ap[1:]))
                nc.sync.dma_start(rb, src)
                state[h] = (cp, rb)
            return fn

        def norm_step(h):
            def fn():
                cp, rb = state[h]
                rc = work.tile([64, 512], f32, tag="rc", name=f"rc{c4}_{h}")
                nc.vector.reciprocal_approx_fast(rc, rb)
                nc.vector.tensor_tensor(
                    xcat[h * 64:(h + 1) * 64, :], cp[0:64, :], rc, op=MUL)
            return fn

        def oproj_step(oc, use_act):
            def fn():
                po = psPX.tile([128, 512], f32, tag="px", name=f"po{c4}_{oc}")
                nc.tensor.matmul(po, lhsT=wmcat_sb[:, oc * 128:(oc + 1) * 128],
                                 rhs=xcat, start=True, stop=True)
                ob = outp.tile([128, 512], f32, tag="ob", name=f"ob{c4}_{oc}")
                if use_act:
                    nc.scalar.copy(ob, po)
                else:
                    nc.vector.tensor_copy(ob, po)
                nc.sync.dma_start(io["out"][oc * 128:(oc + 1) * 128, cs], ob)
            return fn

        return cp_step, norm_step, oproj_step

    for c4 in range(4):
        cs = slice(c4 * 512, (c4 + 1) * 512)
        px = [psPX.tile([128, 512], f32, tag="px", name=f"px{c4}_{h}")
              for h in range(2)]
        ptb = ptp.tile([128, 16, 2, 512], fp8, tag="pt", name=f"pt{c4}")
        xcat = xcp.tile([128, 512], bf16, tag="xc", name=f"xc{c4}")
        cp_step, norm_step, oproj_step = make_tail(c4, px, xcat)

        for mb in range(MB):
            sc = psSC.tile([128, 1024], f32, tag="sc", name=f"sc{c4}_{mb}")
            for h in range(2):
                nc.tensor.matmul(
                    sc[:, h * 512:(h + 1) * 512],
                    lhsT=k_sb[h * 64:(h + 1) * 64, mb * 128:(mb + 1) * 128],
                    rhs=q_sb[h * 64:(h + 1) * 64, cs],
                    start=True, stop=True,
                    tile_position=(h * 64, 0) if CFG_TILEPOS else None,
                )
            j = mb % 8
            use_act = ((j + c4) % 2 == 0)
            if not CFG_SCHRAUD:
                use_act = True
            elif not CFG_ACT_FP8:
                use_act = False
            pslice = ptb[:, mb, :, :]
            if use_act:
                nc.scalar.activation(pslice, sc, EXP, scale=0.125, bias=ln2b)
            else:
                nc.vector.tensor_scalar(
                    pslice.bitcast(i8), sc, SCHRAUD_A, SCHRAUD_B, MUL, ADD)
            if mb >= 8:
                for h in range(2):
                    if CFG_DR:
                        nc.tensor.matmul(
                            px[h][0:80, :],
                            lhsT=vt[:, j, h, :, :],
                            rhs=ptb[:, j::8, h, :],
                            start=(j == 0), stop=(j == 7),
                            perf_mode=DR,
                        )
                    else:
                        for i in range(2):
                            nc.tensor.matmul(
                                px[h][0:65, :],
                                lhsT=vt[:, j, h, i, 0:65],
                                rhs=ptb[:, j + 8 * i, h, :],
                                start=(j == 0 and i == 0),
                                stop=(j == 7 and i == 1),
                            )
            run_deferred((c4, mb))

        # schedule this chunk's tail into the next chunk's loop (or run now
        # for the last chunk)
        tail_ops = [cp_step(0, True), cp_step(1, False), norm_step(0),
                    norm_step(1), oproj_step(0, True), oproj_step(1, False)]
        if c4 < 3:
            for k, fn in enumerate(tail_ops):
                deferred.setdefault((c4 + 1, 1 + 2 * k), []).append(fn)
        else:
            for fn in tail_ops:
                fn()

    if "dbg_q" in io:
        nc.sync.dma_start(io["dbg_q"], q_sb)
        nc.sync.dma_start(io["dbg_k"], k_sb)
        nc.sync.dma_start(io["dbg_vt"], vt.bitcast(i8))


def _build_nc(debug_dumps=False):
    key = ("nc", debug_dumps)
    if key in _CACHE:
        return _CACHE[key]
    from contextlib import ExitStack

    import concourse.mybir as mybir
    import concourse.tile as tile
    from concourse import bacc

    f32 = mybir.dt.float32
    bf16 = mybir.dt.bfloat16
    i8 = mybir.dt.int8
    nc = bacc.Bacc("TRN2", target_bir_lowering=False, debug=False, num_devices=8)
    io = {}
    for name, shape, dt_ in (
        ("xq", [128, 4, 2, 512], bf16),
        ("xk", [128, 4, 2, 512], bf16),
        ("xv", [128, 4, 2, 512], bf16),
        ("wqt", [128, 2, 128], bf16),
        ("wkt", [128, 2, 128], bf16),
        ("wvt", [128, 2, 128], bf16),
        ("bq", [128, 1], f32),
        ("bk", [128, 1], f32),
        ("bv", [1, 128], bf16),
        ("wmcat", [128, 256], bf16),
    ):
        io[name] = nc.dram_tensor(name, shape, dt_, kind="ExternalInput").ap()
    io["out"] = nc.dram_tensor("out", [256, 2048], f32, kind="ExternalOutput").ap()
    if debug_dumps:
        io["dbg_q"] = nc.dram_tensor("dbg_q", [128, 2048], bf16, kind="ExternalOutput").ap()
        io["dbg_k"] = nc.dram_tensor("dbg_k", [128, 2048], bf16, kind="ExternalOutput").ap()
        io["dbg_vt"] = nc.dram_tensor("dbg_vt", [128, 8, 2, 2, 80], i8, kind="ExternalOutput").ap()

    with tile.TileContext(nc) as tc:
        with ExitStack() as ctx:
            _emit(ctx, tc, io)
    nc.compile()
    _CACHE[key] = nc
    _CACHE[(key, "io")] = io
    return nc


def make_in_maps(query, key, value, wq, bq, wk, bk, wv, bv, wm, bm):
    fb = lambda a: np.ascontiguousarray(np.asarray(a, dtype=np.float32).astype(BF))
    f = lambda a: np.ascontiguousarray(np.asarray(a), dtype=np.float32)
    query, key, value = f(query), f(key), f(value)
    wq, wk, wv, wm = f(wq), f(wk), f(wv), f(wm)
    bq, bk, bv = f(bq), f(bk), f(bv)

    def win(x):
        # [256, 2048] -> [128 p, 4 w, 2 cc, 512] with channel = cc*128 + p
        return fb(x.reshape(2, 128, 4, 512).transpose(1, 2, 0, 3))

    def wt(w, idx):
        # [256 in, 128 out(hd)] -> [128 p, 2 cc, 128 o]
        return fb(w[idx].T.reshape(2, 128, 128).transpose(1, 0, 2))

    in_maps = []
    for c in range(8):
        b, pair = divmod(c, 2)
        hs = (2 * pair, 2 * pair + 1)
        idx = np.array([d * H + h for h in hs for d in range(DIM)])
        m = {
            "xq": win(query[b]),
            "xk": win(key[b]),
            "xv": win(value[b]),
            "wqt": wt(wq, idx),
            "wkt": wt(wk, idx),
            "wvt": wt(wv, idx),
            "bq": f(bq[idx].reshape(128, 1)),
            "bk": f(bk[idx].reshape(128, 1)),
            "bv": fb(bv[idx].reshape(1, 128)),
            "wmcat": fb(wm[:, idx].T),
        }
        in_maps.append(m)
    return in_maps


def run(in_maps, trace=False, **kw):
    from concourse import bass_utils

    nc = _build_nc()
    return bass_utils.run_bass_kernel_spmd(
        nc, in_maps, core_ids=list(range(8)), trace=trace, **kw
    )


def gather(results, bm):
    bm = np.asarray(bm, dtype=np.float32)
    outs = [np.asarray(r["out"], dtype=np.float32) for r in results]
    return np.stack([outs[2 * b] + outs[2 * b + 1] + bm[:, None] for b in range(B)])


def kernel(query, key, value, wq, bq, wk, bk, wv, bv, wm, bm):
    in_maps = make_in_maps(query, key, value, wq, bq, wk, bk, wv, bv, wm, bm)
    res = run(in_maps)
    return gather(res.results, bm)
